# revision 24
# baseline (speedup 1.0000x reference)
"""DistMult decoder kernel for 8 Trainium2 NeuronCores.

Computes out = (input1 * weight[type_index]) @ input2.T + bias with
input1 [8192, 512], input2 [8192, 512] in fp32, out [8192, 8192].

Sharding: rows of input1 (and thus rows of the output) are split across
the 8 cores; input2 / weight / bias are replicated. No communication.

Per-core device program (M = 1024 rows):
  - lhsT  [512, 1024]  = w_r-scaled shard of input1, transposed + cast
    to fp16 on host (K-major)
  - rhs   [512, 8192]  = input2 transposed + cast to fp16 on host
  - fp16 operands run the PE at 1 cycle/row (4x fp32) with fp32 PSUM
    accumulation; max-rel error vs the fp32 reference ~3e-4
  - GEMM over n-pairs: 8 groups x 8 m-tiles x (4 k x 2 n) matmuls
  - PSUM -> SBUF copy + bias add on ACT, 512 KB output stores
"""

import os

import numpy as np

import concourse.bacc as bacc
import concourse.mybir as mybir
from concourse.bass_utils import run_bass_kernel_spmd
from concourse.tile import TileContext

N_CORES = 8
N1, N2, D = 8192, 8192, 512
M = N1 // N_CORES  # rows per core
P = 128            # partitions
KT = D // P        # 4 k-tiles
MT = M // P        # 8 m-tiles
NFREE = 512        # psum bank free size (fp32)
NGRP = 1024        # n columns per group (pair of psum banks)
NT = N2 // NGRP    # 8 n-groups

# test.py hooks: set TRACE=True before calling kernel() to profile; the
# BassKernelResults of the last run lands in LAST_RESULTS.
TRACE = os.environ.get("BASS_KERNEL_TRACE", "0") == "1"
LAST_RESULTS = None

_cached_nc = None


def _build():
    nc = bacc.Bacc(
        "TRN2", target_bir_lowering=False, debug=False, enable_asserts=False, num_devices=N_CORES
    )
    f32 = mybir.dt.float32
    f16 = mybir.dt.float16
    lhsT = nc.dram_tensor("lhsT", [D, M], f16, kind="ExternalInput")
    rhs = nc.dram_tensor("rhs", [D, N2], f16, kind="ExternalInput")
    biasv = nc.dram_tensor("biasv", [P, 1], f32, kind="ExternalInput")
    out = nc.dram_tensor("out", [M, N2], f32, kind="ExternalOutput")

    # K-major DRAM views split into [P, KT, cols] for single-DMA loads.
    lhsT_r = lhsT[:, :].rearrange("(kt p) m -> p kt m", p=P)
    rhs_r = rhs[:, :].rearrange("(kt p) n -> p kt n", p=P)

    with TileContext(nc) as tc:
        with (
            tc.tile_pool(name="const", bufs=1) as constp,
            tc.tile_pool(name="lhs", bufs=1) as lhsp,
            tc.tile_pool(name="rhsp", bufs=4) as rhsp,
            tc.tile_pool(name="outp", bufs=8) as outp,
            tc.tile_pool(name="psum", bufs=4, space="PSUM") as psump,
        ):
            # Head: spread the startup loads across all three DGE rings
            # (each ring tops out well below HBM bandwidth) so the PE can
            # start as soon as the preamble ends: Sync carries the first
            # rhs half-group, the Scalar ring carries lhsT, GpSimd carries
            # the second rhs half-group.
            # Ring assignment tuned so arrival order matches consumption
            # order of the first m-tile (ps0 k0..k3 then ps1), with ~2 us
            # DMA completion-receipt latency on every load: Sync carries
            # rt0h0 + k3, Scalar carries k0/k2 + rt0h1, GpSimd carries k1
            # and is then free to start the steady-state rhs prefetch.
            lt = lhsp.tile([P, KT, M], f16, tag="lhs")
            rt0 = rhsp.tile([P, KT, NGRP], f16, tag="rhs")
            nc.sync.dma_start(out=rt0[:, :, 0:NFREE], in_=rhs_r[:, :, 0:NFREE])
            nc.scalar.dma_start(out=lt[:, 0, :], in_=lhsT_r[:, 0, :])
            nc.gpsimd.dma_start(out=lt[:, 1, :], in_=lhsT_r[:, 1, :])
            nc.scalar.dma_start(out=lt[:, 2, :], in_=lhsT_r[:, 2, :])
            nc.sync.dma_start(out=lt[:, 3, :], in_=lhsT_r[:, 3, :])
            nc.scalar.dma_start(
                out=rt0[:, :, NFREE:NGRP], in_=rhs_r[:, :, NFREE:NGRP]
            )
            bias_t = constp.tile([P, 1], f32, tag="bias")
            nc.scalar.dma_start(out=bias_t[:], in_=biasv[:, :])

            # Warm up the PE's HAM clock gate during the head-load window:
            # ~5 us of dummy matmuls on zeroed SBUF (no data deps) push the
            # PE through its 3.4 us busy window so the real matmuls start
            # at 2.4 GHz instead of ramping from 1.2 GHz.
            warm_w = constp.tile([P, P], f16, tag="warmw")
            warm_r = constp.tile([P, NFREE], f16, tag="warmr")
            nc.vector.memset(warm_w[:], 0.0)
            nc.vector.memset(warm_r[:], 0.0)
            wps = psump.tile([P, NFREE], f32, tag="warm", bufs=1)
            NWARM = 16
            for i in range(NWARM):
                nc.tensor.matmul(
                    wps[:], warm_w[:], warm_r[:],
                    start=(i == 0), stop=(i == NWARM - 1),
                )

            # rhs loads run on the GpSimd (SWDGE) queue so they never sit
            # behind output stores in the Sync engine's FIFO; one group of
            # lookahead keeps the PE fed across group boundaries.
            rts = {0: rt0}

            def load_rhs(g):
                rt = rhsp.tile([P, KT, NGRP], f16, tag="rhs")
                nc.gpsimd.dma_start(
                    out=rt[:], in_=rhs_r[:, :, g * NGRP : (g + 1) * NGRP]
                )
                rts[g] = rt

            for n in range(NT):
                rt = rts.pop(n)
                for m in range(MT):
                    # Stagger rhs prefetch into the compute stream, keeping
                    # two groups of lookahead: iter 0 loads groups 1 and 2
                    # (staggered), iter n>=1 tops up with group n+2.
                    if m == 0:
                        if n == 0:
                            load_rhs(1)
                        elif n + 2 < NT:
                            load_rhs(n + 2)
                    if m == 4 and n == 0:
                        load_rhs(2)
                    ps0 = psump.tile([P, NFREE], f32, tag="ps0")
                    ps1 = psump.tile([P, NFREE], f32, tag="ps1", bufs=3)
                    # ps0's k-loop completes before ps1 starts: the copy of
                    # ps0 can begin 3 matmuls earlier, and at kernel start
                    # the PE only waits on the first rhs half-group.
                    for k in range(KT):
                        nc.tensor.matmul(
                            ps0[:], lt[:, k, m * P : (m + 1) * P],
                            rt[:, k, 0:NFREE],
                            start=(k == 0), stop=(k == KT - 1),
                        )
                    for k in range(KT):
                        nc.tensor.matmul(
                            ps1[:], lt[:, k, m * P : (m + 1) * P],
                            rt[:, k, NFREE:NGRP],
                            start=(k == 0), stop=(k == KT - 1),
                        )
                    ot = outp.tile([P, NGRP], f32, tag="ot")
                    # Split psum->sbuf+bias between ACT and the otherwise
                    # idle DVE so neither serializes the psum pool.
                    nc.scalar.activation(
                        ot[:, 0:NFREE], ps0[:],
                        mybir.ActivationFunctionType.Identity,
                        bias=bias_t[:, 0:1],
                    )
                    nc.vector.tensor_scalar_add(
                        ot[:, NFREE:NGRP], ps1[:], bias_t[:, 0:1]
                    )
                    if n == NT - 1 and m == MT - 1:
                        # Final tile: store the halves separately on both
                        # rings so the kernel-exit barrier isn't waiting on
                        # one serial copy+store chain.
                        nc.sync.dma_start(
                            out=out[m * P : (m + 1) * P,
                                    n * NGRP : n * NGRP + NFREE],
                            in_=ot[:, 0:NFREE],
                        )
                        nc.scalar.dma_start(
                            out=out[m * P : (m + 1) * P,
                                    n * NGRP + NFREE : (n + 1) * NGRP],
                            in_=ot[:, NFREE:NGRP],
                        )
                    else:
                        # Alternate stores across the two HWDGE rings so
                        # the store stream drains on both.
                        st = nc.sync if m % 2 == 0 else nc.scalar
                        st.dma_start(
                            out=out[m * P : (m + 1) * P,
                                    n * NGRP : (n + 1) * NGRP],
                            in_=ot[:],
                        )
    nc.compile()
    return nc


def kernel(input1, input2, weight, bias, type_index):
    global _cached_nc, LAST_RESULTS

    input1 = np.asarray(input1, dtype=np.float32)
    input2 = np.asarray(input2, dtype=np.float32)
    weight = np.asarray(weight, dtype=np.float32)
    bias = np.asarray(bias, dtype=np.float32).reshape(-1)
    w_r = weight[int(type_index)]  # [D]

    # Host-side prep: fold the w_r row-scale into input1, lay both GEMM
    # operands out K-major, cast to fp16 (device accumulates in fp32).
    scaled = input1 * w_r[None, :]  # [N1, D]
    rhsT = np.ascontiguousarray(input2.T.astype(np.float16))  # [D, N2]
    bias_vec = np.full((P, 1), float(bias[0]), dtype=np.float32)

    in_maps = []
    for c in range(N_CORES):
        shard = scaled[c * M : (c + 1) * M]  # [M, D]
        in_maps.append(
            {
                "lhsT": np.ascontiguousarray(shard.T.astype(np.float16)),
                "rhs": rhsT,
                "biasv": bias_vec,
            }
        )

    if _cached_nc is None:
        _cached_nc = _build()

    res = run_bass_kernel_spmd(
        _cached_nc, in_maps, core_ids=list(range(N_CORES)), trace=TRACE
    )
    LAST_RESULTS = res
    return np.concatenate([res.results[c]["out"] for c in range(N_CORES)], axis=0)


# revision 26
# speedup vs baseline: 1.0421x; 1.0421x over previous
"""DistMult decoder kernel for 8 Trainium2 NeuronCores.

Computes out = (input1 * weight[type_index]) @ input2.T + bias with
input1 [8192, 512], input2 [8192, 512] in fp32, out [8192, 8192].

Sharding: rows of input1 (and thus rows of the output) are split across
the 8 cores; input2 / weight / bias are replicated. No communication.

Per-core device program (M = 1024 rows):
  - lhsT  [512, 1024]  = w_r-scaled shard of input1, transposed + cast
    to fp16 on host (K-major)
  - rhs   [512, 8192]  = input2 transposed + cast to fp16 on host
  - fp16 operands run the PE at 1 cycle/row (4x fp32) with fp32 PSUM
    accumulation; max-rel error vs the fp32 reference ~3e-4
  - GEMM over n-pairs: 8 groups x 8 m-tiles x (4 k x 2 n) matmuls
  - PSUM -> SBUF copy + bias add on ACT, 512 KB output stores
"""

import os

import numpy as np

import concourse.bacc as bacc
import concourse.mybir as mybir
from concourse.bass_utils import run_bass_kernel_spmd
from concourse.tile import TileContext

N_CORES = 8
N1, N2, D = 8192, 8192, 512
M = N1 // N_CORES  # rows per core
P = 128            # partitions
KT = D // P        # 4 k-tiles
MT = M // P        # 8 m-tiles
NFREE = 512        # psum bank free size (fp32)
NGRP = 1024        # n columns per group (pair of psum banks)
NT = N2 // NGRP    # 8 n-groups

# test.py hooks: set TRACE=True before calling kernel() to profile; the
# BassKernelResults of the last run lands in LAST_RESULTS.
TRACE = os.environ.get("BASS_KERNEL_TRACE", "0") == "1"
LAST_RESULTS = None

_cached_nc = None


def _build():
    nc = bacc.Bacc(
        "TRN2", target_bir_lowering=False, debug=False, enable_asserts=False, num_devices=N_CORES
    )
    f32 = mybir.dt.float32
    f16 = mybir.dt.float16
    lhsT = nc.dram_tensor("lhsT", [D, M], f16, kind="ExternalInput")
    rhs = nc.dram_tensor("rhs", [D, N2], f16, kind="ExternalInput")
    biasv = nc.dram_tensor("biasv", [P, 1], f32, kind="ExternalInput")
    out = nc.dram_tensor("out", [M, N2], f32, kind="ExternalOutput")

    # K-major DRAM views split into [P, KT, cols] for single-DMA loads.
    lhsT_r = lhsT[:, :].rearrange("(kt p) m -> p kt m", p=P)
    rhs_r = rhs[:, :].rearrange("(kt p) n -> p kt n", p=P)

    with TileContext(nc) as tc:
        with (
            tc.tile_pool(name="const", bufs=1) as constp,
            tc.tile_pool(name="lhs", bufs=1) as lhsp,
            tc.tile_pool(name="rhsp", bufs=4) as rhsp,
            tc.tile_pool(name="outp", bufs=8) as outp,
            tc.tile_pool(name="psum", bufs=4, space="PSUM") as psump,
        ):
            # Head: spread the startup loads across all three DGE rings
            # (each ring tops out well below HBM bandwidth) so the PE can
            # start as soon as the preamble ends: Sync carries the first
            # rhs half-group, the Scalar ring carries lhsT, GpSimd carries
            # the second rhs half-group.
            lt = lhsp.tile([P, KT, M], f16, tag="lhs")
            rt0 = rhsp.tile([P, KT, NGRP], f16, tag="rhs")
            nc.sync.dma_start(out=rt0[:, :, 0:NFREE], in_=rhs_r[:, :, 0:NFREE])
            nc.scalar.dma_start(out=lt[:, 0, :], in_=lhsT_r[:, 0, :])
            nc.gpsimd.dma_start(out=lt[:, 1, :], in_=lhsT_r[:, 1, :])
            nc.scalar.dma_start(out=lt[:, 2, :], in_=lhsT_r[:, 2, :])
            nc.sync.dma_start(out=lt[:, 3, :], in_=lhsT_r[:, 3, :])
            bias_t = constp.tile([P, 1], f32, tag="bias")
            nc.scalar.dma_start(out=bias_t[:], in_=biasv[:, :])
            nc.gpsimd.dma_start(
                out=rt0[:, :, NFREE:NGRP], in_=rhs_r[:, :, NFREE:NGRP]
            )

            # Warm up the PE's HAM clock gate during the head-load window:
            # ~5 us of dummy matmuls on zeroed SBUF (no data deps) push the
            # PE through its 3.4 us busy window so the real matmuls start
            # at 2.4 GHz instead of ramping from 1.2 GHz.
            warm_w = constp.tile([P, P], f16, tag="warmw")
            warm_r = constp.tile([P, NFREE], f16, tag="warmr")
            nc.vector.memset(warm_w[:], 0.0)
            nc.vector.memset(warm_r[:], 0.0)
            wps = psump.tile([P, NFREE], f32, tag="warm", bufs=1)
            NWARM = 16
            for i in range(NWARM):
                nc.tensor.matmul(
                    wps[:], warm_w[:], warm_r[:],
                    start=(i == 0), stop=(i == NWARM - 1),
                )

            # rhs loads run on the GpSimd (SWDGE) queue so they never sit
            # behind output stores in the Sync engine's FIFO; one group of
            # lookahead keeps the PE fed across group boundaries.
            rts = {0: rt0}

            def load_rhs(g):
                rt = rhsp.tile([P, KT, NGRP], f16, tag="rhs")
                nc.gpsimd.dma_start(
                    out=rt[:], in_=rhs_r[:, :, g * NGRP : (g + 1) * NGRP]
                )
                rts[g] = rt

            for n in range(NT):
                rt = rts.pop(n)
                for m in range(MT):
                    # Stagger rhs prefetch into the compute stream, keeping
                    # two groups of lookahead: iter 0 loads groups 1 and 2
                    # (staggered), iter n>=1 tops up with group n+2.
                    if m == 0:
                        if n == 0:
                            load_rhs(1)
                        elif n + 2 < NT:
                            load_rhs(n + 2)
                    if m == 4 and n == 0:
                        load_rhs(2)
                    ps0 = psump.tile([P, NFREE], f32, tag="ps0")
                    ps1 = psump.tile([P, NFREE], f32, tag="ps1", bufs=3)
                    # ps0's k-loop completes before ps1 starts: the copy of
                    # ps0 can begin 3 matmuls earlier, and at kernel start
                    # the PE only waits on the first rhs half-group.
                    for k in range(KT):
                        nc.tensor.matmul(
                            ps0[:], lt[:, k, m * P : (m + 1) * P],
                            rt[:, k, 0:NFREE],
                            start=(k == 0), stop=(k == KT - 1),
                        )
                    for k in range(KT):
                        nc.tensor.matmul(
                            ps1[:], lt[:, k, m * P : (m + 1) * P],
                            rt[:, k, NFREE:NGRP],
                            start=(k == 0), stop=(k == KT - 1),
                        )
                    ot = outp.tile([P, NGRP], f32, tag="ot")
                    # Split psum->sbuf+bias between ACT and the otherwise
                    # idle DVE so neither serializes the psum pool.
                    nc.scalar.activation(
                        ot[:, 0:NFREE], ps0[:],
                        mybir.ActivationFunctionType.Identity,
                        bias=bias_t[:, 0:1],
                    )
                    nc.vector.tensor_scalar_add(
                        ot[:, NFREE:NGRP], ps1[:], bias_t[:, 0:1]
                    )
                    # Alternate stores across the two HWDGE rings so the
                    # store stream drains on both and the tail is shorter.
                    st = nc.sync if m % 2 == 0 else nc.scalar
                    st.dma_start(
                        out=out[m * P : (m + 1) * P, n * NGRP : (n + 1) * NGRP],
                        in_=ot[:],
                    )
    nc.compile()
    return nc


def kernel(input1, input2, weight, bias, type_index):
    global _cached_nc, LAST_RESULTS

    input1 = np.asarray(input1, dtype=np.float32)
    input2 = np.asarray(input2, dtype=np.float32)
    weight = np.asarray(weight, dtype=np.float32)
    bias = np.asarray(bias, dtype=np.float32).reshape(-1)
    w_r = weight[int(type_index)]  # [D]

    # Host-side prep: fold the w_r row-scale into input1, lay both GEMM
    # operands out K-major, cast to fp16 (device accumulates in fp32).
    scaled = input1 * w_r[None, :]  # [N1, D]
    rhsT = np.ascontiguousarray(input2.T.astype(np.float16))  # [D, N2]
    bias_vec = np.full((P, 1), float(bias[0]), dtype=np.float32)

    in_maps = []
    for c in range(N_CORES):
        shard = scaled[c * M : (c + 1) * M]  # [M, D]
        in_maps.append(
            {
                "lhsT": np.ascontiguousarray(shard.T.astype(np.float16)),
                "rhs": rhsT,
                "biasv": bias_vec,
            }
        )

    if _cached_nc is None:
        _cached_nc = _build()

    res = run_bass_kernel_spmd(
        _cached_nc, in_maps, core_ids=list(range(N_CORES)), trace=TRACE
    )
    LAST_RESULTS = res
    return np.concatenate([res.results[c]["out"] for c in range(N_CORES)], axis=0)


# revision 27
# speedup vs baseline: 1.0445x; 1.0023x over previous
"""DistMult decoder kernel for 8 Trainium2 NeuronCores.

Computes out = (input1 * weight[type_index]) @ input2.T + bias with
input1 [8192, 512], input2 [8192, 512] in fp32, out [8192, 8192].

Sharding: rows of input1 (and thus rows of the output) are split across
the 8 cores; input2 / weight / bias are replicated. No communication.

Per-core device program (M = 1024 rows):
  - lhsT  [512, 1024]  = w_r-scaled shard of input1, transposed + cast
    to fp16 on host (K-major)
  - rhs   [512, 8192]  = input2 transposed + cast to fp16 on host
  - fp16 operands run the PE at 1 cycle/row (4x fp32) with fp32 PSUM
    accumulation; max-rel error vs the fp32 reference ~3e-4
  - GEMM over n-pairs: 8 groups x 8 m-tiles x (4 k x 2 n) matmuls
  - PSUM -> SBUF copy + bias add on ACT, 512 KB output stores
"""

import os

import numpy as np

import concourse.bacc as bacc
import concourse.mybir as mybir
from concourse.bass_utils import run_bass_kernel_spmd
from concourse.tile import TileContext

N_CORES = 8
N1, N2, D = 8192, 8192, 512
M = N1 // N_CORES  # rows per core
P = 128            # partitions
KT = D // P        # 4 k-tiles
MT = M // P        # 8 m-tiles
NFREE = 512        # psum bank free size (fp32)
NGRP = 1024        # n columns per group (pair of psum banks)
NT = N2 // NGRP    # 8 n-groups

# test.py hooks: set TRACE=True before calling kernel() to profile; the
# BassKernelResults of the last run lands in LAST_RESULTS.
TRACE = os.environ.get("BASS_KERNEL_TRACE", "0") == "1"
LAST_RESULTS = None

_cached_nc = None


def _build():
    nc = bacc.Bacc(
        "TRN2", target_bir_lowering=False, debug=False, enable_asserts=False, num_devices=N_CORES
    )
    f32 = mybir.dt.float32
    f16 = mybir.dt.float16
    lhsT = nc.dram_tensor("lhsT", [D, M], f16, kind="ExternalInput")
    rhs = nc.dram_tensor("rhs", [D, N2], f16, kind="ExternalInput")
    biasv = nc.dram_tensor("biasv", [P, 1], f32, kind="ExternalInput")
    out = nc.dram_tensor("out", [M, N2], f32, kind="ExternalOutput")

    # K-major DRAM views split into [P, KT, cols] for single-DMA loads.
    lhsT_r = lhsT[:, :].rearrange("(kt p) m -> p kt m", p=P)
    rhs_r = rhs[:, :].rearrange("(kt p) n -> p kt n", p=P)

    with TileContext(nc) as tc:
        with (
            tc.tile_pool(name="const", bufs=1) as constp,
            tc.tile_pool(name="lhs", bufs=1) as lhsp,
            tc.tile_pool(name="rhsp", bufs=4) as rhsp,
            tc.tile_pool(name="outp", bufs=8) as outp,
            tc.tile_pool(name="psum", bufs=4, space="PSUM") as psump,
        ):
            # Head: spread the startup loads across all three DGE rings
            # (each ring tops out well below HBM bandwidth) so the PE can
            # start as soon as the preamble ends: Sync carries the first
            # rhs half-group, the Scalar ring carries lhsT, GpSimd carries
            # the second rhs half-group.
            lt = lhsp.tile([P, KT, M], f16, tag="lhs")
            rt0 = rhsp.tile([P, KT, NGRP], f16, tag="rhs")
            nc.sync.dma_start(out=rt0[:, :, 0:NFREE], in_=rhs_r[:, :, 0:NFREE])
            nc.scalar.dma_start(out=lt[:, 0, :], in_=lhsT_r[:, 0, :])
            nc.gpsimd.dma_start(out=lt[:, 1, :], in_=lhsT_r[:, 1, :])
            nc.scalar.dma_start(out=lt[:, 2, :], in_=lhsT_r[:, 2, :])
            nc.sync.dma_start(out=lt[:, 3, :], in_=lhsT_r[:, 3, :])
            bias_t = constp.tile([P, 1], f32, tag="bias")
            nc.scalar.dma_start(out=bias_t[:], in_=biasv[:, :])
            nc.gpsimd.dma_start(
                out=rt0[:, :, NFREE:NGRP], in_=rhs_r[:, :, NFREE:NGRP]
            )

            # Warm up the PE's HAM clock gate during the head-load window:
            # ~5 us of dummy matmuls on zeroed SBUF (no data deps) push the
            # PE through its 3.4 us busy window so the real matmuls start
            # at 2.4 GHz instead of ramping from 1.2 GHz.
            warm_w = constp.tile([P, P], f16, tag="warmw")
            warm_r = constp.tile([P, NFREE], f16, tag="warmr")
            nc.vector.memset(warm_w[:], 0.0)
            nc.vector.memset(warm_r[:], 0.0)
            wps = psump.tile([P, NFREE], f32, tag="warm", bufs=1)
            NWARM = 16
            for i in range(NWARM):
                nc.tensor.matmul(
                    wps[:], warm_w[:], warm_r[:],
                    start=(i == 0), stop=(i == NWARM - 1),
                )

            # rhs loads run on the GpSimd (SWDGE) queue so they never sit
            # behind output stores in the Sync engine's FIFO; one group of
            # lookahead keeps the PE fed across group boundaries.
            rts = {0: rt0}

            def load_rhs(g):
                rt = rhsp.tile([P, KT, NGRP], f16, tag="rhs")
                nc.gpsimd.dma_start(
                    out=rt[:], in_=rhs_r[:, :, g * NGRP : (g + 1) * NGRP]
                )
                rts[g] = rt

            for n in range(NT):
                rt = rts.pop(n)
                for m in range(MT):
                    # Stagger rhs prefetch into the compute stream, keeping
                    # two groups of lookahead: iter 0 loads groups 1 and 2
                    # (staggered), iter n>=1 tops up with group n+2.
                    if m == 0:
                        if n == 0:
                            load_rhs(1)
                        elif n + 2 < NT:
                            load_rhs(n + 2)
                    if m == 4 and n == 0:
                        load_rhs(2)
                    ps0 = psump.tile([P, NFREE], f32, tag="ps0")
                    ps1 = psump.tile([P, NFREE], f32, tag="ps1", bufs=3)
                    # ps0's k-loop completes before ps1 starts: the copy of
                    # ps0 can begin 3 matmuls earlier, and at kernel start
                    # the PE only waits on the first rhs half-group.
                    for k in range(KT):
                        nc.tensor.matmul(
                            ps0[:], lt[:, k, m * P : (m + 1) * P],
                            rt[:, k, 0:NFREE],
                            start=(k == 0), stop=(k == KT - 1),
                        )
                    for k in range(KT):
                        nc.tensor.matmul(
                            ps1[:], lt[:, k, m * P : (m + 1) * P],
                            rt[:, k, NFREE:NGRP],
                            start=(k == 0), stop=(k == KT - 1),
                        )
                    ot = outp.tile([P, NGRP], f32, tag="ot")
                    # Split psum->sbuf+bias between ACT and the otherwise
                    # idle DVE so neither serializes the psum pool.
                    nc.scalar.activation(
                        ot[:, 0:NFREE], ps0[:],
                        mybir.ActivationFunctionType.Identity,
                        bias=bias_t[:, 0:1],
                    )
                    nc.vector.tensor_scalar_add(
                        ot[:, NFREE:NGRP], ps1[:], bias_t[:, 0:1]
                    )
                    if n == NT - 1 and m == MT - 1:
                        # Final tile: store the halves separately on both
                        # rings so the kernel-exit barrier isn't waiting on
                        # one serial copy+store chain.
                        nc.sync.dma_start(
                            out=out[m * P : (m + 1) * P,
                                    n * NGRP : n * NGRP + NFREE],
                            in_=ot[:, 0:NFREE],
                        )
                        nc.scalar.dma_start(
                            out=out[m * P : (m + 1) * P,
                                    n * NGRP + NFREE : (n + 1) * NGRP],
                            in_=ot[:, NFREE:NGRP],
                        )
                    else:
                        # Alternate stores across the two HWDGE rings so
                        # the store stream drains on both.
                        st = nc.sync if m % 2 == 0 else nc.scalar
                        st.dma_start(
                            out=out[m * P : (m + 1) * P,
                                    n * NGRP : (n + 1) * NGRP],
                            in_=ot[:],
                        )
    nc.compile()
    return nc


def kernel(input1, input2, weight, bias, type_index):
    global _cached_nc, LAST_RESULTS

    input1 = np.asarray(input1, dtype=np.float32)
    input2 = np.asarray(input2, dtype=np.float32)
    weight = np.asarray(weight, dtype=np.float32)
    bias = np.asarray(bias, dtype=np.float32).reshape(-1)
    w_r = weight[int(type_index)]  # [D]

    # Host-side prep: fold the w_r row-scale into input1, lay both GEMM
    # operands out K-major, cast to fp16 (device accumulates in fp32).
    scaled = input1 * w_r[None, :]  # [N1, D]
    rhsT = np.ascontiguousarray(input2.T.astype(np.float16))  # [D, N2]
    bias_vec = np.full((P, 1), float(bias[0]), dtype=np.float32)

    in_maps = []
    for c in range(N_CORES):
        shard = scaled[c * M : (c + 1) * M]  # [M, D]
        in_maps.append(
            {
                "lhsT": np.ascontiguousarray(shard.T.astype(np.float16)),
                "rhs": rhsT,
                "biasv": bias_vec,
            }
        )

    if _cached_nc is None:
        _cached_nc = _build()

    res = run_bass_kernel_spmd(
        _cached_nc, in_maps, core_ids=list(range(N_CORES)), trace=TRACE
    )
    LAST_RESULTS = res
    return np.concatenate([res.results[c]["out"] for c in range(N_CORES)], axis=0)
